# revision 10
# baseline (speedup 1.0000x reference)
"""Grouped-Query Attention (B=2, S=2048, D=2048, H=32, KV=8, HD=64) on 8 TRN2
NeuronCores, tensor-parallel over KV-head groups (1 KV head + 4 Q heads per
core), with host-side shard/gather.

Per-core dataflow (activations kept feature-on-partitions so every matmul
contracts over the partition dim with no on-device transposition of x):

  phase 1  QKV projection + RoPE
    xT[d-tile, tok-tile] (DMA) -> psum: qa = wqa.T@xT, qb = wqb.T@xT,
    kv = [ka|kb|v].T@xT;  RoPE on DVE directly from PSUM; V transposed back to
    natural [tok, hd] via PE transpose (PV matmul lhsT needs it).
  phase 2  attention per (batch, q-tile of 512), causal-block-skipped
    scoresT[sk=128, q=512] = krot.T @ qrot (heads packed 2-per-PE-pass via row
    groups);  probsT = exp(scale*scoresT) (ACT, no max-subtraction: |s|<=6
    verified on the actual distribution);  diagonal blocks masked by a 0/1
    mask multiply;  PV accumulates outT[65, 512] = [1|V].T @ probsT over
    sk-tiles (row 0 = softmax denominator via the ones column).
  phase 3  output projection y[tok, 512] = attn_outT.T @ wo, DMA out.

Host sums the 8 per-core partial y (wo is row-sharded).
"""

import contextlib
import os
import numpy as np

import concourse.bass as bass
import concourse.tile as tile
from concourse import bacc, mybir
from concourse.bass_utils import run_bass_kernel_spmd
from concourse.masks import make_identity

B, S, D = 2, 2048, 2048
H, KV, HD = 32, 8, 64
T = B * S
NCORES = 8
HPC = H // NCORES          # 4 query heads per core
SCALE = 1.0 / np.sqrt(HD)
THETA = 10000.0
NQT = T // 512             # 8 token tiles of 512
NDT = D // 128             # 16 contraction tiles
F32 = mybir.dt.float32

# fp32r: 4-byte fp32 storage, reduced-precision full-rate matmul (1 cyc/row at
# free-dim >= 256 vs 4 for strict fp32).  Flip to "0" to fall back.
USE_F32R = os.environ.get("GQA_F32R", "1") == "1"
MM_DT = mybir.dt.float32r if USE_F32R else mybir.dt.float32


def _bc(ap):
    # DRAM-side view for DMA into an MM_DT tile (bit-identical 4-byte cast)
    return ap.bitcast(MM_DT) if USE_F32R else ap


def _build_program():
    nc = bacc.Bacc("TRN2", target_bir_lowering=False, debug=False)

    xT = nc.dram_tensor("xT", [D, T], F32, kind="ExternalInput")
    wq = nc.dram_tensor("wq", [D, 2 * HPC * 32], F32, kind="ExternalInput")
    wkv = nc.dram_tensor("wkv", [D, 128], F32, kind="ExternalInput")
    wo = nc.dram_tensor("wo", [HPC * HD, D], F32, kind="ExternalInput")
    cos4 = nc.dram_tensor("cos4", [128, S], F32, kind="ExternalInput")
    sin4 = nc.dram_tensor("sin4", [128, S], F32, kind="ExternalInput")
    maskd = nc.dram_tensor("mask", [128, 896], F32, kind="ExternalInput")
    y = nc.dram_tensor("y", [T, D], F32, kind="ExternalOutput")

    with tile.TileContext(nc) as tc:
        _body(tc, nc, xT, wq, wkv, wo, cos4, sin4, maskd, y)
    nc.compile()
    return nc


def _body(tc, nc, xT, wq, wkv, wo, cos4, sin4, maskd, y):
    TT = mybir.AluOpType
    SC_NAMES = ["ps_a", "ps_b", "ps_c", "ps_t"]
    ctx = contextlib.ExitStack()
    with ctx:
        const = ctx.enter_context(tc.tile_pool(name="const", bufs=1))
        persist = ctx.enter_context(tc.tile_pool(name="persist", bufs=1))
        xs = ctx.enter_context(tc.tile_pool(name="xs", bufs=3))
        rtmp = ctx.enter_context(tc.tile_pool(name="rtmp", bufs=1))
        probs = ctx.enter_context(tc.tile_pool(name="probs", bufs=1))
        norm = ctx.enter_context(tc.tile_pool(name="norm", bufs=1))
        yout = ctx.enter_context(tc.tile_pool(name="yout", bufs=2))
        # PSUM is 8 banks of [128 x 512 f32].  Tags: ps_a/b/c/t shared across
        # phases (proj accumulators -> score tiles -> out-proj), pv0-3 are the
        # PV accumulators.  Total static reservation = exactly 8 banks.
        psum = ctx.enter_context(tc.tile_pool(name="psum", bufs=1, space="PSUM"))

        # ---- constants ----
        wq_sb = const.tile([128, NDT, 256], MM_DT, name="wq_sb")
        nc.sync.dma_start(out=wq_sb, in_=_bc(wq[:, :].rearrange("(t p) c -> p t c", p=128)))
        wkv_sb = const.tile([128, NDT, 128], MM_DT, name="wkv_sb")
        nc.sync.dma_start(out=wkv_sb, in_=_bc(wkv[:, :].rearrange("(t p) c -> p t c", p=128)))
        wo_sb = const.tile([128, 2, D], MM_DT, name="wo_sb")
        nc.sync.dma_start(out=wo_sb, in_=_bc(wo[:, :].rearrange("(t p) c -> p t c", p=128)))
        cos_sb = const.tile([128, S], F32, name="cos_sb")
        nc.sync.dma_start(out=cos_sb, in_=cos4[:, :])
        sin_sb = const.tile([128, S], F32, name="sin_sb")
        nc.sync.dma_start(out=sin_sb, in_=sin4[:, :])
        mask_sb = const.tile([128, 896], MM_DT, name="mask_sb")
        nc.sync.dma_start(out=mask_sb, in_=_bc(maskd[:, :]))
        ident = const.tile([64, 64], F32, name="ident")
        make_identity(nc, ident)

        # ---- persistent activations ----
        qrot = persist.tile([128, 2, T], MM_DT, name="qrot")   # [rowgrp, headpair, tok]
        krot2 = persist.tile([128, T], MM_DT, name="krot2")    # rows 64:128 = copy of 0:64
        vnat = persist.tile([128, T // 128, 65], MM_DT, name="vnat")  # [tok%128, toktile, hd+1]
        a0 = persist.tile([128, T], MM_DT, name="a0")          # attn outT, heads 0,1
        a1 = persist.tile([128, T], MM_DT, name="a1")          # attn outT, heads 2,3
        ones_c = const.tile([128, T // 128, 1], F32, name="ones_c")
        nc.vector.memset(ones_c, 1.0)
        nc.vector.tensor_copy(out=vnat[:, :, 64:65], in_=ones_c)

        # ================= phase 1: projections + rope =================
        for qt in range(NQT):
            pos0 = (qt % 4) * 512
            tok0 = qt * 512
            qa_ps = psum.tile([128, 512], F32, name="ps_a")
            qb_ps = psum.tile([128, 512], F32, name="ps_b")
            kv_ps = psum.tile([128, 512], F32, name="ps_c")
            for d in range(NDT):
                xt = xs.tile([128, 512], MM_DT, name="xt")
                nc.sync.dma_start(out=xt, in_=_bc(xT[d * 128:(d + 1) * 128, tok0:tok0 + 512]))
                st, sp = d == 0, d == NDT - 1
                nc.tensor.matmul(out=qa_ps, lhsT=(wq_sb[:, d, 0:128]), rhs=(xt),
                                 start=st, stop=sp)
                nc.tensor.matmul(out=qb_ps, lhsT=(wq_sb[:, d, 128:256]), rhs=(xt),
                                 start=st, stop=sp)
                nc.tensor.matmul(out=kv_ps, lhsT=(wkv_sb[:, d, :]), rhs=(xt),
                                 start=st, stop=sp)
            cs = cos_sb[:, pos0:pos0 + 512]
            sn = sin_sb[:, pos0:pos0 + 512]
            # Q rope on [128, 512] (row 32h+r = head h dim r); both reads of
            # each psum issued back-to-back so the bank frees early.
            t_x = rtmp.tile([128, 512], F32, name="t_x")
            t_x2 = rtmp.tile([128, 512], F32, name="t_x2")
            nc.vector.tensor_tensor(out=t_x, in0=qa_ps, in1=cs, op=TT.mult)
            nc.vector.tensor_tensor(out=t_x2, in0=qa_ps, in1=sn, op=TT.mult)
            t_y = rtmp.tile([128, 512], F32, name="t_y")
            t_y2 = rtmp.tile([128, 512], F32, name="t_y2")
            nc.vector.tensor_tensor(out=t_y, in0=qb_ps, in1=sn, op=TT.mult)
            nc.vector.tensor_tensor(out=t_y2, in0=qb_ps, in1=cs, op=TT.mult)
            qra = rtmp.tile([128, 512], MM_DT, name="qra")
            qrb = rtmp.tile([128, 512], MM_DT, name="qrb")
            nc.vector.tensor_tensor(out=qra, in0=t_x, in1=t_y, op=TT.subtract)
            nc.vector.tensor_tensor(out=qrb, in0=t_x2, in1=t_y2, op=TT.add)
            # remap into [rowgrp(h%2), headpair(h//2)] layout for packed scores
            for h in range(HPC):
                rb = (h % 2) * 64
                blk = h // 2
                nc.sync.dma_start(out=qrot[rb:rb + 32, blk, tok0:tok0 + 512],
                                  in_=qra[32 * h:32 * h + 32, :])
                nc.sync.dma_start(out=qrot[rb + 32:rb + 64, blk, tok0:tok0 + 512],
                                  in_=qrb[32 * h:32 * h + 32, :])
            # K rope (single kv head): rows 0:32 ka, 32:64 kb of kv_ps; V copy.
            k_x = rtmp.tile([32, 512], F32, name="k_x")
            k_x2 = rtmp.tile([32, 512], F32, name="k_x2")
            k_y = rtmp.tile([32, 512], F32, name="k_y")
            k_y2 = rtmp.tile([32, 512], F32, name="k_y2")
            vt = rtmp.tile([64, 512], F32, name="vt")
            nc.vector.tensor_tensor(out=k_x, in0=kv_ps[0:32], in1=cs[0:32], op=TT.mult)
            nc.vector.tensor_tensor(out=k_x2, in0=kv_ps[0:32], in1=sn[0:32], op=TT.mult)
            nc.vector.tensor_tensor(out=k_y, in0=kv_ps[32:64], in1=sn[0:32], op=TT.mult)
            nc.vector.tensor_tensor(out=k_y2, in0=kv_ps[32:64], in1=cs[0:32], op=TT.mult)
            nc.vector.tensor_copy(out=vt, in_=kv_ps[64:128])
            nc.vector.tensor_tensor(out=krot2[0:32, tok0:tok0 + 512], in0=k_x,
                                    in1=k_y, op=TT.subtract)
            nc.vector.tensor_tensor(out=krot2[32:64, tok0:tok0 + 512], in0=k_x2,
                                    in1=k_y2, op=TT.add)
            # V back to natural layout [tok, hd] via PE transpose
            for k4 in range(4):
                tp = psum.tile([128, 64], F32, name="ps_t")
                nc.tensor.transpose(tp, vt[:, k4 * 128:(k4 + 1) * 128], ident)
                nc.vector.tensor_copy(out=vnat[:, qt * 4 + k4, 0:64], in_=tp)

        # replicate krot rows 0:64 -> 64:128 so head pairs pack into row groups
        nc.sync.dma_start(out=krot2[64:128, :], in_=krot2[0:64, :])

        # ================= phase 2: attention =================
        for b in range(B):
            for jq in range(4):
                tq = b * S + jq * 512
                pv = [psum.tile([65, 512], F32, name=f"ps_pv{h}") for h in range(HPC)]
                ni = 4 * jq + 4
                for i in range(ni):
                    tk = b * S + i * 128
                    sc = [psum.tile([128, 512], F32, name=SC_NAMES[h])
                          for h in range(HPC)]
                    for h in range(HPC):
                        rb = (h % 2) * 64
                        blk = h // 2
                        nc.tensor.matmul(
                            out=sc[h],
                            lhsT=(krot2[rb:rb + 64, tk:tk + 128]),
                            rhs=(qrot[rb:rb + 64, blk, tq:tq + 512]),
                            start=True, stop=True)
                    for h in range(HPC):
                        pt = probs.tile([128, 512], MM_DT, name=f"pt{h}")
                        nc.scalar.activation(out=pt, in_=sc[h],
                                             func=mybir.ActivationFunctionType.Exp,
                                             scale=float(SCALE))
                        if i >= 4 * jq:  # diagonal block: causal mask
                            roff = 128 * i - 512 * jq
                            nc.vector.tensor_tensor(
                                out=pt, in0=pt,
                                in1=mask_sb[:, 384 - roff:896 - roff], op=TT.mult)
                        nc.tensor.matmul(out=pv[h], lhsT=(vnat[:, b * 16 + i, :]),
                                         rhs=(pt), start=(i == 0), stop=(i == ni - 1))
                # normalize: row 64 of pv[h] is the softmax denominator
                sums = norm.tile([1, HPC * 512], F32, name="sums")
                for h in range(HPC):
                    nc.scalar.copy(out=sums[0:1, h * 512:(h + 1) * 512],
                                   in_=pv[h][64:65])
                rec = norm.tile([1, HPC * 512], F32, name="rec")
                nc.vector.reciprocal(out=rec, in_=sums)
                for h in range(HPC):
                    rbc = norm.tile([64, 512], F32, name="rbc")
                    nc.gpsimd.partition_broadcast(rbc, rec[0:1, h * 512:(h + 1) * 512])
                    dst = a0 if h < 2 else a1
                    rb = (h % 2) * 64
                    nc.vector.tensor_tensor(out=dst[rb:rb + 64, tq:tq + 512],
                                            in0=pv[h][0:64], in1=rbc, op=TT.mult)

        # ================= phase 3: output projection =================
        for tt in range(T // 128):
            for n in range(D // 512):
                yo = psum.tile([128, 512], F32, name=SC_NAMES[(tt * 4 + n) % 4])
                nc.tensor.matmul(out=yo, lhsT=(a0[:, tt * 128:(tt + 1) * 128]),
                                 rhs=(wo_sb[:, 0, n * 512:(n + 1) * 512]),
                                 start=True, stop=False)
                nc.tensor.matmul(out=yo, lhsT=(a1[:, tt * 128:(tt + 1) * 128]),
                                 rhs=(wo_sb[:, 1, n * 512:(n + 1) * 512]),
                                 start=False, stop=True)
                ys = yout.tile([128, 512], F32, name="ys")
                if n % 2 == 0:
                    nc.scalar.copy(out=ys, in_=yo)
                else:
                    nc.vector.tensor_copy(out=ys, in_=yo)
                nc.sync.dma_start(out=y[tt * 128:(tt + 1) * 128, n * 512:(n + 1) * 512],
                                  in_=ys)


_CACHE = {}


def _get_program():
    if "nc" not in _CACHE:
        _CACHE["nc"] = _build_program()
    return _CACHE["nc"]


def _get_runner():
    """Cached jitted shard_map executable over 8 cores (avoids per-call
    retrace that run_bass_kernel_spmd pays)."""
    if "runner" in _CACHE:
        return _CACHE["runner"]
    import jax
    from jax.sharding import Mesh, PartitionSpec
    from jax.experimental.shard_map import shard_map
    from concourse import bass2jax
    from concourse.bass2jax import _bass_exec_p

    bass2jax.install_neuronx_cc_hook()
    nc = _get_program()
    partition_name = nc.partition_id_tensor.name if nc.partition_id_tensor else None
    in_names, out_names, out_avals = [], [], []
    for alloc in nc.m.functions[0].allocations:
        if not isinstance(alloc, mybir.MemoryLocationSet):
            continue
        name = alloc.memorylocations[0].name
        if alloc.kind == "ExternalInput":
            if name != partition_name:
                in_names.append(name)
        elif alloc.kind == "ExternalOutput":
            out_names.append(name)
            out_avals.append(jax.core.ShapedArray(
                tuple(alloc.tensor_shape), mybir.dt.np(alloc.dtype)))
    n_params = len(in_names)
    n_outs = len(out_avals)
    all_in = list(in_names) + list(out_names)
    if partition_name is not None:
        all_in.append(partition_name)

    def _body(*args):
        operands = list(args)
        if partition_name is not None:
            operands.append(bass2jax.partition_id_tensor())
        return tuple(_bass_exec_p.bind(
            *operands,
            out_avals=tuple(out_avals),
            in_names=tuple(all_in),
            out_names=tuple(out_names),
            lowering_input_output_aliases=(),
            sim_require_finite=True,
            sim_require_nnan=True,
            nc=nc,
        ))

    devices = jax.devices()[:NCORES]
    mesh = Mesh(np.asarray(devices), ("core",))
    sharded = jax.jit(
        shard_map(_body, mesh=mesh,
                  in_specs=(PartitionSpec("core"),) * (n_params + n_outs),
                  out_specs=(PartitionSpec("core"),) * n_outs,
                  check_rep=False),
        donate_argnums=tuple(range(n_params, n_params + n_outs)),
        keep_unused=True)
    _CACHE["runner"] = (sharded, in_names, out_names, out_avals)
    return _CACHE["runner"]


def _host_inputs(x, wq, wk, wv, wo):
    x = np.asarray(x, np.float32)
    wq = np.asarray(wq, np.float32)
    wk = np.asarray(wk, np.float32)
    wv = np.asarray(wv, np.float32)
    wo = np.asarray(wo, np.float32)

    xT = np.ascontiguousarray(x.reshape(T, D).T)

    inv = 1.0 / (THETA ** (np.arange(0, HD, 2, dtype=np.float64) / HD))
    fr = np.outer(inv, np.arange(S, dtype=np.float64))   # [32, S]
    cosT = np.cos(fr).astype(np.float32)
    sinT = np.sin(fr).astype(np.float32)
    cos4 = np.ascontiguousarray(np.tile(cosT, (4, 1)))
    sin4 = np.ascontiguousarray(np.tile(sinT, (4, 1)))

    u = np.arange(896)[None, :]
    p = np.arange(128)[:, None]
    mask = (u >= p + 384).astype(np.float32)

    in_maps = []
    for c in range(NCORES):
        cols_a, cols_b = [], []
        for h in range(HPC):
            base = (HPC * c + h) * HD
            cols_a.append(wq[:, base:base + 32])
            cols_b.append(wq[:, base + 32:base + 64])
        wq_c = np.ascontiguousarray(np.concatenate(cols_a + cols_b, axis=1))
        kb = c * HD
        wkv_c = np.ascontiguousarray(np.concatenate(
            [wk[:, kb:kb + 32], wk[:, kb + 32:kb + 64], wv[:, kb:kb + HD]], axis=1))
        wo_c = np.ascontiguousarray(wo[c * HPC * HD:(c + 1) * HPC * HD, :])
        in_maps.append({"xT": xT, "wq": wq_c, "wkv": wkv_c, "wo": wo_c,
                        "cos4": cos4, "sin4": sin4, "mask": mask})
    return in_maps


def kernel(x, wq, wk, wv, wo):
    import jax
    sharded, in_names, out_names, out_avals = _get_runner()
    in_maps = _host_inputs(x, wq, wk, wv, wo)
    concat_in = [np.concatenate([m[n] for m in in_maps], axis=0) for n in in_names]
    concat_zeros = [np.zeros((NCORES * a.shape[0], *a.shape[1:]), a.dtype)
                    for a in out_avals]
    out_arrs = sharded(*concat_in, *concat_zeros)
    ycat = np.asarray(out_arrs[out_names.index("y")])  # [8*T, D]
    out = ycat.reshape(NCORES, T, D).sum(axis=0, dtype=np.float64)
    return out.astype(np.float32).reshape(B, S, D)


# revision 11
# speedup vs baseline: 169.7880x; 169.7880x over previous
"""Grouped-Query Attention (B=2, S=2048, D=2048, H=32, KV=8, HD=64) on 8 TRN2
NeuronCores, tensor-parallel over KV-head groups (1 KV head + 4 Q heads per
core), with host-side shard/gather.

Per-core dataflow (activations kept feature-on-partitions so every matmul
contracts over the partition dim with no on-device transposition of x):

  phase 1  QKV projection + RoPE
    xT[d-tile, tok-tile] (DMA) -> psum: qa = wqa.T@xT, qb = wqb.T@xT,
    kv = [ka|kb|v].T@xT;  RoPE on DVE directly from PSUM; V transposed back to
    natural [tok, hd] via PE transpose (PV matmul lhsT needs it).
  phase 2  attention per (batch, q-tile of 512), causal-block-skipped
    scoresT[sk=128, q=512] = krot.T @ qrot (heads packed 2-per-PE-pass via row
    groups);  probsT = exp(scale*scoresT) (ACT, no max-subtraction: |s|<=6
    verified on the actual distribution);  diagonal blocks masked by a 0/1
    mask multiply;  PV accumulates outT[65, 512] = [1|V].T @ probsT over
    sk-tiles (row 0 = softmax denominator via the ones column).
  phase 3  output projection y[tok, 512] = attn_outT.T @ wo, DMA out.

Host sums the 8 per-core partial y (wo is row-sharded).
"""

import contextlib
import os
import numpy as np
import jax.numpy as jnp

import concourse.bass as bass
import concourse.tile as tile
from concourse import bacc, mybir
from concourse.bass_utils import run_bass_kernel_spmd
from concourse.masks import make_identity

B, S, D = 2, 2048, 2048
H, KV, HD = 32, 8, 64
T = B * S
NCORES = 8
HPC = H // NCORES          # 4 query heads per core
SCALE = 1.0 / np.sqrt(HD)
THETA = 10000.0
NQT = T // 512             # 8 token tiles of 512
REPLICATED = {"xT", "cos4", "sin4", "mask"}  # same bytes on every core
NDT = D // 128             # 16 contraction tiles
F32 = mybir.dt.float32

# fp32r: 4-byte fp32 storage, reduced-precision full-rate matmul (1 cyc/row at
# free-dim >= 256 vs 4 for strict fp32).  Flip to "0" to fall back.
USE_F32R = os.environ.get("GQA_F32R", "1") == "1"
MM_DT = mybir.dt.float32r if USE_F32R else mybir.dt.float32


def _bc(ap):
    # DRAM-side view for DMA into an MM_DT tile (bit-identical 4-byte cast)
    return ap.bitcast(MM_DT) if USE_F32R else ap


def _build_program():
    nc = bacc.Bacc("TRN2", target_bir_lowering=False, debug=False)

    xT = nc.dram_tensor("xT", [D, T], F32, kind="ExternalInput")
    wq = nc.dram_tensor("wq", [D, 2 * HPC * 32], F32, kind="ExternalInput")
    wkv = nc.dram_tensor("wkv", [D, 128], F32, kind="ExternalInput")
    wo = nc.dram_tensor("wo", [HPC * HD, D], F32, kind="ExternalInput")
    cos4 = nc.dram_tensor("cos4", [128, S], F32, kind="ExternalInput")
    sin4 = nc.dram_tensor("sin4", [128, S], F32, kind="ExternalInput")
    maskd = nc.dram_tensor("mask", [128, 896], F32, kind="ExternalInput")
    y = nc.dram_tensor("y", [T, D], F32, kind="ExternalOutput")

    with tile.TileContext(nc) as tc:
        _body(tc, nc, xT, wq, wkv, wo, cos4, sin4, maskd, y)
    nc.compile()
    return nc


def _body(tc, nc, xT, wq, wkv, wo, cos4, sin4, maskd, y):
    TT = mybir.AluOpType
    SC_NAMES = ["ps_a", "ps_b", "ps_c", "ps_t"]
    ctx = contextlib.ExitStack()
    with ctx:
        const = ctx.enter_context(tc.tile_pool(name="const", bufs=1))
        persist = ctx.enter_context(tc.tile_pool(name="persist", bufs=1))
        xs = ctx.enter_context(tc.tile_pool(name="xs", bufs=3))
        rtmp = ctx.enter_context(tc.tile_pool(name="rtmp", bufs=1))
        probs = ctx.enter_context(tc.tile_pool(name="probs", bufs=1))
        norm = ctx.enter_context(tc.tile_pool(name="norm", bufs=1))
        yout = ctx.enter_context(tc.tile_pool(name="yout", bufs=2))
        # PSUM is 8 banks of [128 x 512 f32].  Tags: ps_a/b/c/t shared across
        # phases (proj accumulators -> score tiles -> out-proj), pv0-3 are the
        # PV accumulators.  Total static reservation = exactly 8 banks.
        psum = ctx.enter_context(tc.tile_pool(name="psum", bufs=1, space="PSUM"))

        # ---- constants ----
        wq_sb = const.tile([128, NDT, 256], MM_DT, name="wq_sb")
        nc.sync.dma_start(out=wq_sb, in_=_bc(wq[:, :].rearrange("(t p) c -> p t c", p=128)))
        wkv_sb = const.tile([128, NDT, 128], MM_DT, name="wkv_sb")
        nc.sync.dma_start(out=wkv_sb, in_=_bc(wkv[:, :].rearrange("(t p) c -> p t c", p=128)))
        wo_sb = const.tile([128, 2, D], MM_DT, name="wo_sb")
        nc.sync.dma_start(out=wo_sb, in_=_bc(wo[:, :].rearrange("(t p) c -> p t c", p=128)))
        cos_sb = const.tile([128, S], F32, name="cos_sb")
        nc.sync.dma_start(out=cos_sb, in_=cos4[:, :])
        sin_sb = const.tile([128, S], F32, name="sin_sb")
        nc.sync.dma_start(out=sin_sb, in_=sin4[:, :])
        mask_sb = const.tile([128, 896], MM_DT, name="mask_sb")
        nc.sync.dma_start(out=mask_sb, in_=_bc(maskd[:, :]))
        ident = const.tile([64, 64], F32, name="ident")
        make_identity(nc, ident)

        # ---- persistent activations ----
        qrot = persist.tile([128, 2, T], MM_DT, name="qrot")   # [rowgrp, headpair, tok]
        krot2 = persist.tile([128, T], MM_DT, name="krot2")    # rows 64:128 = copy of 0:64
        vnat = persist.tile([128, T // 128, 65], MM_DT, name="vnat")  # [tok%128, toktile, hd+1]
        a0 = persist.tile([128, T], MM_DT, name="a0")          # attn outT, heads 0,1
        a1 = persist.tile([128, T], MM_DT, name="a1")          # attn outT, heads 2,3
        ones_c = const.tile([128, T // 128, 1], F32, name="ones_c")
        nc.vector.memset(ones_c, 1.0)
        nc.vector.tensor_copy(out=vnat[:, :, 64:65], in_=ones_c)

        # ================= phase 1: projections + rope =================
        for qt in range(NQT):
            pos0 = (qt % 4) * 512
            tok0 = qt * 512
            qa_ps = psum.tile([128, 512], F32, name="ps_a")
            qb_ps = psum.tile([128, 512], F32, name="ps_b")
            kv_ps = psum.tile([128, 512], F32, name="ps_c")
            for d in range(NDT):
                xt = xs.tile([128, 512], MM_DT, name="xt")
                nc.sync.dma_start(out=xt, in_=_bc(xT[d * 128:(d + 1) * 128, tok0:tok0 + 512]))
                st, sp = d == 0, d == NDT - 1
                nc.tensor.matmul(out=qa_ps, lhsT=(wq_sb[:, d, 0:128]), rhs=(xt),
                                 start=st, stop=sp)
                nc.tensor.matmul(out=qb_ps, lhsT=(wq_sb[:, d, 128:256]), rhs=(xt),
                                 start=st, stop=sp)
                nc.tensor.matmul(out=kv_ps, lhsT=(wkv_sb[:, d, :]), rhs=(xt),
                                 start=st, stop=sp)
            cs = cos_sb[:, pos0:pos0 + 512]
            sn = sin_sb[:, pos0:pos0 + 512]
            # Q rope on [128, 512] (row 32h+r = head h dim r); both reads of
            # each psum issued back-to-back so the bank frees early.
            t_x = rtmp.tile([128, 512], F32, name="t_x")
            t_x2 = rtmp.tile([128, 512], F32, name="t_x2")
            nc.vector.tensor_tensor(out=t_x, in0=qa_ps, in1=cs, op=TT.mult)
            nc.vector.tensor_tensor(out=t_x2, in0=qa_ps, in1=sn, op=TT.mult)
            t_y = rtmp.tile([128, 512], F32, name="t_y")
            t_y2 = rtmp.tile([128, 512], F32, name="t_y2")
            nc.vector.tensor_tensor(out=t_y, in0=qb_ps, in1=sn, op=TT.mult)
            nc.vector.tensor_tensor(out=t_y2, in0=qb_ps, in1=cs, op=TT.mult)
            qra = rtmp.tile([128, 512], MM_DT, name="qra")
            qrb = rtmp.tile([128, 512], MM_DT, name="qrb")
            nc.vector.tensor_tensor(out=qra, in0=t_x, in1=t_y, op=TT.subtract)
            nc.vector.tensor_tensor(out=qrb, in0=t_x2, in1=t_y2, op=TT.add)
            # remap into [rowgrp(h%2), headpair(h//2)] layout for packed scores
            for h in range(HPC):
                rb = (h % 2) * 64
                blk = h // 2
                nc.sync.dma_start(out=qrot[rb:rb + 32, blk, tok0:tok0 + 512],
                                  in_=qra[32 * h:32 * h + 32, :])
                nc.sync.dma_start(out=qrot[rb + 32:rb + 64, blk, tok0:tok0 + 512],
                                  in_=qrb[32 * h:32 * h + 32, :])
            # K rope (single kv head): rows 0:32 ka, 32:64 kb of kv_ps; V copy.
            k_x = rtmp.tile([32, 512], F32, name="k_x")
            k_x2 = rtmp.tile([32, 512], F32, name="k_x2")
            k_y = rtmp.tile([32, 512], F32, name="k_y")
            k_y2 = rtmp.tile([32, 512], F32, name="k_y2")
            vt = rtmp.tile([64, 512], F32, name="vt")
            nc.vector.tensor_tensor(out=k_x, in0=kv_ps[0:32], in1=cs[0:32], op=TT.mult)
            nc.vector.tensor_tensor(out=k_x2, in0=kv_ps[0:32], in1=sn[0:32], op=TT.mult)
            nc.vector.tensor_tensor(out=k_y, in0=kv_ps[32:64], in1=sn[0:32], op=TT.mult)
            nc.vector.tensor_tensor(out=k_y2, in0=kv_ps[32:64], in1=cs[0:32], op=TT.mult)
            nc.vector.tensor_copy(out=vt, in_=kv_ps[64:128])
            nc.vector.tensor_tensor(out=krot2[0:32, tok0:tok0 + 512], in0=k_x,
                                    in1=k_y, op=TT.subtract)
            nc.vector.tensor_tensor(out=krot2[32:64, tok0:tok0 + 512], in0=k_x2,
                                    in1=k_y2, op=TT.add)
            # V back to natural layout [tok, hd] via PE transpose
            for k4 in range(4):
                tp = psum.tile([128, 64], F32, name="ps_t")
                nc.tensor.transpose(tp, vt[:, k4 * 128:(k4 + 1) * 128], ident)
                nc.vector.tensor_copy(out=vnat[:, qt * 4 + k4, 0:64], in_=tp)

        # replicate krot rows 0:64 -> 64:128 so head pairs pack into row groups
        nc.sync.dma_start(out=krot2[64:128, :], in_=krot2[0:64, :])

        # ================= phase 2: attention =================
        for b in range(B):
            for jq in range(4):
                tq = b * S + jq * 512
                pv = [psum.tile([65, 512], F32, name=f"ps_pv{h}") for h in range(HPC)]
                ni = 4 * jq + 4
                for i in range(ni):
                    tk = b * S + i * 128
                    sc = [psum.tile([128, 512], F32, name=SC_NAMES[h])
                          for h in range(HPC)]
                    for h in range(HPC):
                        rb = (h % 2) * 64
                        blk = h // 2
                        nc.tensor.matmul(
                            out=sc[h],
                            lhsT=(krot2[rb:rb + 64, tk:tk + 128]),
                            rhs=(qrot[rb:rb + 64, blk, tq:tq + 512]),
                            start=True, stop=True)
                    for h in range(HPC):
                        pt = probs.tile([128, 512], MM_DT, name=f"pt{h}")
                        nc.scalar.activation(out=pt, in_=sc[h],
                                             func=mybir.ActivationFunctionType.Exp,
                                             scale=float(SCALE))
                        if i >= 4 * jq:  # diagonal block: causal mask
                            roff = 128 * i - 512 * jq
                            nc.vector.tensor_tensor(
                                out=pt, in0=pt,
                                in1=mask_sb[:, 384 - roff:896 - roff], op=TT.mult)
                        nc.tensor.matmul(out=pv[h], lhsT=(vnat[:, b * 16 + i, :]),
                                         rhs=(pt), start=(i == 0), stop=(i == ni - 1))
                # normalize: row 64 of pv[h] is the softmax denominator
                sums = norm.tile([1, HPC * 512], F32, name="sums")
                for h in range(HPC):
                    nc.scalar.copy(out=sums[0:1, h * 512:(h + 1) * 512],
                                   in_=pv[h][64:65])
                rec = norm.tile([1, HPC * 512], F32, name="rec")
                nc.vector.reciprocal(out=rec, in_=sums)
                for h in range(HPC):
                    rbc = norm.tile([64, 512], F32, name="rbc")
                    nc.gpsimd.partition_broadcast(rbc, rec[0:1, h * 512:(h + 1) * 512])
                    dst = a0 if h < 2 else a1
                    rb = (h % 2) * 64
                    nc.vector.tensor_tensor(out=dst[rb:rb + 64, tq:tq + 512],
                                            in0=pv[h][0:64], in1=rbc, op=TT.mult)

        # ================= phase 3: output projection =================
        for tt in range(T // 128):
            for n in range(D // 512):
                yo = psum.tile([128, 512], F32, name=SC_NAMES[(tt * 4 + n) % 4])
                nc.tensor.matmul(out=yo, lhsT=(a0[:, tt * 128:(tt + 1) * 128]),
                                 rhs=(wo_sb[:, 0, n * 512:(n + 1) * 512]),
                                 start=True, stop=False)
                nc.tensor.matmul(out=yo, lhsT=(a1[:, tt * 128:(tt + 1) * 128]),
                                 rhs=(wo_sb[:, 1, n * 512:(n + 1) * 512]),
                                 start=False, stop=True)
                ys = yout.tile([128, 512], F32, name="ys")
                if n % 2 == 0:
                    nc.scalar.copy(out=ys, in_=yo)
                else:
                    nc.vector.tensor_copy(out=ys, in_=yo)
                nc.sync.dma_start(out=y[tt * 128:(tt + 1) * 128, n * 512:(n + 1) * 512],
                                  in_=ys)


_CACHE = {}


def _get_program():
    if "nc" not in _CACHE:
        _CACHE["nc"] = _build_program()
    return _CACHE["nc"]


def _get_runner():
    """Cached jitted shard_map executable over 8 cores (avoids per-call
    retrace that run_bass_kernel_spmd pays)."""
    if "runner" in _CACHE:
        return _CACHE["runner"]
    import jax
    from jax.sharding import Mesh, PartitionSpec
    from jax.experimental.shard_map import shard_map
    from concourse import bass2jax
    from concourse.bass2jax import _bass_exec_p

    bass2jax.install_neuronx_cc_hook()
    nc = _get_program()
    partition_name = nc.partition_id_tensor.name if nc.partition_id_tensor else None
    in_names, out_names, out_avals = [], [], []
    for alloc in nc.m.functions[0].allocations:
        if not isinstance(alloc, mybir.MemoryLocationSet):
            continue
        name = alloc.memorylocations[0].name
        if alloc.kind == "ExternalInput":
            if name != partition_name:
                in_names.append(name)
        elif alloc.kind == "ExternalOutput":
            out_names.append(name)
            out_avals.append(jax.core.ShapedArray(
                tuple(alloc.tensor_shape), mybir.dt.np(alloc.dtype)))
    n_params = len(in_names)
    n_outs = len(out_avals)
    all_in = list(in_names) + list(out_names)
    if partition_name is not None:
        all_in.append(partition_name)

    def _body(*args):
        operands = list(args)
        if partition_name is not None:
            operands.append(bass2jax.partition_id_tensor())
        return tuple(_bass_exec_p.bind(
            *operands,
            out_avals=tuple(out_avals),
            in_names=tuple(all_in),
            out_names=tuple(out_names),
            lowering_input_output_aliases=(),
            sim_require_finite=True,
            sim_require_nnan=True,
            nc=nc,
        ))

    devices = jax.devices()[:NCORES]
    mesh = Mesh(np.asarray(devices), ("core",))
    # xT / rope tables / mask are identical on every core: feed them
    # replicated (P()) so the host uploads one copy + on-device all-gather,
    # instead of 8 copies through the tunnel.
    in_specs = tuple(
        PartitionSpec() if n in REPLICATED else PartitionSpec("core")
        for n in in_names) + (PartitionSpec("core"),) * n_outs
    sharded = jax.jit(
        shard_map(_body, mesh=mesh,
                  in_specs=in_specs,
                  out_specs=(PartitionSpec("core"),) * n_outs,
                  check_rep=False),
        donate_argnums=tuple(range(n_params, n_params + n_outs)),
        keep_unused=True)

    from jax.sharding import NamedSharding
    rep = NamedSharding(mesh, PartitionSpec())
    shd = NamedSharding(mesh, PartitionSpec("core"))
    gather = jax.jit(lambda a: a, out_shardings=rep)   # upload-shard -> all-gather
    zeros = jax.jit(lambda: jnp.zeros((NCORES * T, D), jnp.float32),
                    out_shardings=shd)
    reduce_y = jax.jit(lambda yc: yc.reshape(NCORES, T, D)
                       .sum(0, dtype=jnp.float32), out_shardings=rep)
    _CACHE["runner"] = (sharded, in_names, out_names, out_avals,
                        mesh, rep, shd, gather, zeros, reduce_y)
    return _CACHE["runner"]


def _host_inputs(x, wq, wk, wv, wo):
    x = np.asarray(x, np.float32)
    wq = np.asarray(wq, np.float32)
    wk = np.asarray(wk, np.float32)
    wv = np.asarray(wv, np.float32)
    wo = np.asarray(wo, np.float32)

    xT = np.ascontiguousarray(x.reshape(T, D).T)

    inv = 1.0 / (THETA ** (np.arange(0, HD, 2, dtype=np.float64) / HD))
    fr = np.outer(inv, np.arange(S, dtype=np.float64))   # [32, S]
    cosT = np.cos(fr).astype(np.float32)
    sinT = np.sin(fr).astype(np.float32)
    cos4 = np.ascontiguousarray(np.tile(cosT, (4, 1)))
    sin4 = np.ascontiguousarray(np.tile(sinT, (4, 1)))

    u = np.arange(896)[None, :]
    p = np.arange(128)[:, None]
    mask = (u >= p + 384).astype(np.float32)

    in_maps = []
    for c in range(NCORES):
        cols_a, cols_b = [], []
        for h in range(HPC):
            base = (HPC * c + h) * HD
            cols_a.append(wq[:, base:base + 32])
            cols_b.append(wq[:, base + 32:base + 64])
        wq_c = np.ascontiguousarray(np.concatenate(cols_a + cols_b, axis=1))
        kb = c * HD
        wkv_c = np.ascontiguousarray(np.concatenate(
            [wk[:, kb:kb + 32], wk[:, kb + 32:kb + 64], wv[:, kb:kb + HD]], axis=1))
        wo_c = np.ascontiguousarray(wo[c * HPC * HD:(c + 1) * HPC * HD, :])
        in_maps.append({"xT": xT, "wq": wq_c, "wkv": wkv_c, "wo": wo_c,
                        "cos4": cos4, "sin4": sin4, "mask": mask})
    return in_maps


def _stage_inputs(in_maps):
    """Upload inputs: replicated tensors go up as 1/8 shards and are
    all-gathered on device; per-core tensors upload as the usual concat."""
    import jax
    (sharded, in_names, out_names, out_avals,
     mesh, rep, shd, gather, zeros, reduce_y) = _get_runner()
    staged = []
    for n in in_names:
        if n in REPLICATED:
            a = in_maps[0][n]
            if a.shape[0] % NCORES == 0:
                staged.append(gather(jax.device_put(a, shd)))
            else:
                staged.append(jax.device_put(a, rep))
        else:
            cat = np.concatenate([m[n] for m in in_maps], axis=0)
            staged.append(jax.device_put(cat, shd))
    return staged


def kernel(x, wq, wk, wv, wo):
    import jax
    (sharded, in_names, out_names, out_avals,
     mesh, rep, shd, gather, zeros, reduce_y) = _get_runner()
    in_maps = _host_inputs(x, wq, wk, wv, wo)
    staged = _stage_inputs(in_maps)
    out_arrs = sharded(*staged, zeros())
    ysum = reduce_y(out_arrs[out_names.index("y")])
    return np.asarray(ysum).reshape(B, S, D)
